# revision 14
# baseline (speedup 1.0000x reference)
"""Bahdanau-attention-audio kernel for 8 Trainium2 NeuronCores.

Math (reference):
    conv with spatial width 1 degenerates to the center tap:
        convo[b,o] = sum_i prev_att[b,i] * conv_w[o,i,K]
    sum1[b,s,i] = (values @ W1.T)[b,s,i] + W1_b[i] + (q @ W2.T)[b,i] + W2_b[i]
                  + convo[b,s]*locproj[i]
    score[b,s]  = tanh(sum1) @ V_w.T + V_b
    top-100 of score kept per row, rest zeroed; sig = sigmoid(masked);
    aw = sig / sig.sum(batch); ctx[b,h] = sum_s aw[b,s]*values[b,s,h]

Distribution: H_out (rows of W1/W2, i.e. the hidden dim of sum1) is sharded
8 ways; every core streams the full `values`.  Each core produces a partial
score (its 512-slice of the tanh reduction); one 16KB AllReduce yields the
full score on every core, after which top-k/sigmoid/normalize run locally
(replicated) and each core computes its 512-wide slice of the context.

Precision: the big matmul runs as a 3-pass bf16 hi/lo split
(hi@hi + hi@lo + lo@hi with fp32 PSUM accumulation), giving ~4e-6 rms error
on sum1 — required because the smallest top-k boundary gap in the score is
5.1e-5 and a flipped boundary element changes the score output by ~19%.
The small V reduction runs in native fp32.

Host-side prep is limited to dtype splitting (fp32 -> bf16 hi+lo), layout
transposes, slicing per core, and the constant one-hot batch indicator.
"""

import os
from contextlib import ExitStack
import numpy as np
import ml_dtypes

import concourse.bass as bass
import concourse.tile as tile
from concourse import bacc, mybir
from concourse.bass_utils import run_bass_kernel_spmd

F32 = mybir.dt.float32
BF16 = mybir.dt.bfloat16
AF = mybir.ActivationFunctionType
ALU = mybir.AluOpType

B, S, H, TOPK = 20, 198, 4096, 100
BS = B * S            # 3960
BSP = 4096            # padded bs
NCORES = 8
ISLC = H // NCORES    # 512 : per-core H_out slice
NT = BSP // 512       # 8 bs tiles
JC = H // 128         # 32 contraction chunks
NEG = -1.0e30

_cache = {}


def _build(use_collective=True):
    nc = bacc.Bacc("TRN2", target_bir_lowering=False, debug=False,
                   num_devices=NCORES)

    def din(name, shape, dt):
        return nc.dram_tensor(name, shape, dt, kind="ExternalInput").ap()

    vh = din("vh", [H, BSP], BF16)         # valuesT hi (shared)
    vl = din("vl", [H, BSP], BF16)
    w1th = din("w1th", [H, ISLC], BF16)    # W1.T slice
    w1tl = din("w1tl", [H, ISLC], BF16)
    w2th = din("w2th", [H, ISLC], BF16)
    w2tl = din("w2tl", [H, ISLC], BF16)
    qth = din("qth", [H, B], BF16)
    qtl = din("qtl", [H, B], BF16)
    pvh = din("pvh", [S, B], BF16)         # prev_att.T
    pvl = din("pvl", [S, B], BF16)
    cwh = din("cwh", [S, S], BF16)         # cw15.T  (cw15T[i,o] = conv_w[o,i,K])
    cwl = din("cwl", [S, S], BF16)
    lph = din("lph", [1, ISLC], BF16)      # locproj slice hi/lo
    lpl = din("lpl", [1, ISLC], BF16)
    w1b = din("w1b", [1, ISLC], F32)
    w2b = din("w2b", [1, ISLC], F32)
    vwt = din("vwt", [ISLC, 1], F32)       # V_w slice, column
    vb = din("vb", [1, 1], F32)
    oh = din("oh", [B + 2, BSP], BF16)     # one-hot rows + two ones rows
    vch = din("vch", [ISLC, BSP], BF16)    # valuesT rows of MY slice (context)
    vcl = din("vcl", [ISLC, BSP], BF16)

    ctx_o = nc.dram_tensor("ctx_o", [B, ISLC], F32, kind="ExternalOutput").ap()
    aw_o = nc.dram_tensor("aw_o", [B, S], F32, kind="ExternalOutput").ap()
    sc_o = nc.dram_tensor("sc_o", [B, S], F32, kind="ExternalOutput").ap()

    with tile.TileContext(nc) as tc, ExitStack() as stk:
        cst = stk.enter_context(tc.tile_pool(name="cst", bufs=1))
        pp = stk.enter_context(tc.tile_pool(name="pp", bufs=2, space="PSUM"))
        dr = stk.enter_context(tc.tile_pool(name="dr", bufs=1, space="DRAM"))

        # ---- resident W1T as two big tiles (released after main loop) ----
        w1pool = tc.alloc_tile_pool(name="w1pool", bufs=1)
        w1h_all = w1pool.tile([128, JC * ISLC], BF16, tag="w1h_all", name="w1h_all")
        w1l_all = w1pool.tile([128, JC * ISLC], BF16, tag="w1l_all", name="w1l_all")
        for g in range(4):  # 4 DMAs of 1MB each per tensor
            gsl = slice(g * 8 * ISLC, (g + 1) * 8 * ISLC)
            nc.sync.dma_start(
                w1h_all[:, gsl].rearrange("p (g c) -> p g c", g=8),
                w1th[g * 1024:(g + 1) * 1024, :]
                    .rearrange("(g p) c -> p g c", p=128))
            nc.scalar.dma_start(
                w1l_all[:, gsl].rearrange("p (g c) -> p g c", g=8),
                w1tl[g * 1024:(g + 1) * 1024, :]
                    .rearrange("(g p) c -> p g c", p=128))

        # ---- bias_q = q @ W2.T  (out [B, ISLC]) ----
        ps_b = pp.tile([B, ISLC], F32, tag="ppx")
        qh_all = cst.tile([128, JC * B], BF16, tag="qh_all")
        nc.gpsimd.dma_start(qh_all[:].rearrange("p (g c) -> p g c", g=JC),
                            qth[:, :].rearrange("(g p) c -> p g c", p=128))
        ql_all = cst.tile([128, JC * B], BF16, tag="ql_all")
        nc.gpsimd.dma_start(ql_all[:].rearrange("p (g c) -> p g c", g=JC),
                            qtl[:, :].rearrange("(g p) c -> p g c", p=128))
        with tc.tile_pool(name="w2pool", bufs=3) as w2p:
            for jg in range(JC // 4):
                w2h4 = w2p.tile([128, 4 * ISLC], BF16, tag="w2h4", name="w2h4")
                nc.sync.dma_start(
                    w2h4[:].rearrange("p (g c) -> p g c", g=4),
                    w2th[jg * 512:(jg + 1) * 512, :]
                        .rearrange("(g p) c -> p g c", p=128))
                w2l4 = w2p.tile([128, 4 * ISLC], BF16, tag="w2l4", name="w2l4")
                nc.scalar.dma_start(
                    w2l4[:].rearrange("p (g c) -> p g c", g=4),
                    w2tl[jg * 512:(jg + 1) * 512, :]
                        .rearrange("(g p) c -> p g c", p=128))
                for g in range(4):
                    jc = jg * 4 + g
                    qh_t = qh_all[:, jc * B:(jc + 1) * B]
                    ql_t = ql_all[:, jc * B:(jc + 1) * B]
                    wh_t = w2h4[:, g * ISLC:(g + 1) * ISLC]
                    wl_t = w2l4[:, g * ISLC:(g + 1) * ISLC]
                    st = (jc == 0)
                    sp = (jc == JC - 1)
                    nc.tensor.matmul(ps_b[:], qh_t, wh_t, start=st, stop=False)
                    nc.tensor.matmul(ps_b[:], qh_t, wl_t, start=False, stop=False)
                    nc.tensor.matmul(ps_b[:], ql_t, wh_t, start=False, stop=sp)
        bias_sb = cst.tile([B, ISLC], F32, tag="bias_sb")
        nc.scalar.activation(bias_sb[:], ps_b[:], AF.Copy)
        bqh = cst.tile([B, ISLC], BF16, tag="bqh")
        nc.vector.tensor_copy(bqh[:], bias_sb[:])
        bql = cst.tile([B, ISLC], BF16, tag="bql")
        nc.vector.tensor_sub(bql[:], bias_sb[:], bqh[:])

        # ---- bvec = W1_b + W2_b (slice) ----
        w1b_sb = cst.tile([1, ISLC], F32, tag="w1b_sb")
        nc.sync.dma_start(w1b_sb[:], w1b[:, :])
        w2b_sb = cst.tile([1, ISLC], F32, tag="w2b_sb")
        nc.sync.dma_start(w2b_sb[:], w2b[:, :])
        bvec = cst.tile([1, ISLC], F32, tag="bvec")
        nc.vector.tensor_add(bvec[:], w1b_sb[:], w2b_sb[:])
        bvh = cst.tile([1, ISLC], BF16, tag="bvh")
        nc.vector.tensor_copy(bvh[:], bvec[:])
        bvl = cst.tile([1, ISLC], BF16, tag="bvl")
        nc.vector.tensor_sub(bvl[:], bvec[:], bvh[:])

        # ---- convo = prev_att @ cw15.T  (out [B, S]) ----
        ps_c = pp.tile([B, S], F32, tag="ppx")
        ic_sizes = [(0, 128), (128, S - 128)]
        for k, (i0, isz) in enumerate(ic_sizes):
            pvh_t = cst.tile([isz, B], BF16, tag=f"pvh{k}")
            nc.sync.dma_start(pvh_t[:], pvh[i0:i0 + isz, :])
            pvl_t = cst.tile([isz, B], BF16, tag=f"pvl{k}")
            nc.sync.dma_start(pvl_t[:], pvl[i0:i0 + isz, :])
            cwh_t = cst.tile([isz, S], BF16, tag=f"cwh{k}")
            nc.sync.dma_start(cwh_t[:], cwh[i0:i0 + isz, :])
            cwl_t = cst.tile([isz, S], BF16, tag=f"cwl{k}")
            nc.sync.dma_start(cwl_t[:], cwl[i0:i0 + isz, :])
            st = (k == 0)
            sp = (k == len(ic_sizes) - 1)
            nc.tensor.matmul(ps_c[:], pvh_t[:], cwh_t[:], start=st, stop=False)
            nc.tensor.matmul(ps_c[:], pvh_t[:], cwl_t[:], start=False, stop=False)
            nc.tensor.matmul(ps_c[:], pvl_t[:], cwh_t[:], start=False, stop=sp)
        convo_sb = cst.tile([B, S], F32, tag="convo_sb")
        nc.scalar.activation(convo_sb[:], ps_c[:], AF.Copy)
        # flatten via DRAM bounce
        cscr = dr.tile([B, S], F32, tag="cscr")
        nc.sync.dma_start(cscr[:], convo_sb[:])
        cflat = cst.tile([NT, 512], F32, tag="cflat")
        nc.vector.memset(cflat[:], 0.0)
        cflat_full = BS // 512            # 7 full partitions
        cflat_rem = BS - cflat_full * 512  # 376
        nc.sync.dma_start(
            cflat[0:cflat_full, :],
            cscr[:].rearrange("b s -> (b s)")[0:cflat_full * 512]
                .rearrange("(n x) -> n x", x=512))
        nc.sync.dma_start(
            cflat[cflat_full:cflat_full + 1, 0:cflat_rem],
            cscr[:].rearrange("b s -> (b s)")[cflat_full * 512:BS].unsqueeze(0))
        cfh = cst.tile([NT, 512], BF16, tag="cfh")
        nc.vector.tensor_copy(cfh[:], cflat[:])
        cfl = cst.tile([NT, 512], BF16, tag="cfl")
        nc.vector.tensor_sub(cfl[:], cflat[:], cfh[:])

        # ---- lhsT_x [45, ISLC] ----
        KX = 45
        lhsx = cst.tile([KX, ISLC], BF16, tag="lhsx")
        nc.vector.tensor_copy(lhsx[0:B, :], bqh[:])
        nc.gpsimd.dma_start(lhsx[B:2 * B, :], bql[:])
        nc.gpsimd.dma_start(lhsx[40:41, :], lph[:, :])
        nc.gpsimd.dma_start(lhsx[41:42, :], lph[:, :])
        nc.gpsimd.dma_start(lhsx[42:43, :], lpl[:, :])
        nc.gpsimd.dma_start(lhsx[43:44, :], bvh[:])
        nc.gpsimd.dma_start(lhsx[44:45, :], bvl[:])

        # ---- rhs_x per n-tile [45, 512] ----
        rhsx = []
        for n in range(NT):
            t = cst.tile([KX, 512], BF16, tag=f"rhsx{n}")
            sl = slice(n * 512, (n + 1) * 512)
            nc.gpsimd.dma_start(t[0:B, :], oh[0:B, sl])
            nc.gpsimd.dma_start(t[B:2 * B, :], oh[0:B, sl])
            nc.gpsimd.dma_start(t[40:41, :], cfh[n:n + 1, :])
            nc.gpsimd.dma_start(t[41:42, :], cfl[n:n + 1, :])
            nc.gpsimd.dma_start(t[42:43, :], cfh[n:n + 1, :])
            nc.gpsimd.dma_start(t[43:45, :], oh[B:B + 2, sl])
            rhsx.append(t)

        # ---- V_w columns, V_b ----
        vwt_sb = []
        for ib in range(4):
            t = cst.tile([128, 1], F32, tag=f"vwt{ib}")
            nc.sync.dma_start(t[:], vwt[ib * 128:(ib + 1) * 128, :])
            vwt_sb.append(t)
        vb_sb = cst.tile([1, 1], F32, tag="vb_sb")
        nc.sync.dma_start(vb_sb[:], vb[:, :])
        vb8 = cst.tile([1, 1], F32, tag="vb8")
        nc.vector.tensor_scalar_mul(vb8[:], vb_sb[:], 1.0 / NCORES)

        # identity for small PE transposes
        from concourse.masks import make_identity
        ident = cst.tile([128, 128], F32, tag="ident")
        make_identity(nc, ident[:])


        arin = dr.tile([1, BSP], F32, tag="arin", name="arin")
        arout = dr.tile([1, BSP], F32, tag="arout", name="arout")

        # ================= main loop =================
        with tc.tile_pool(name="vpool", bufs=4) as vp, \
             tc.tile_pool(name="mps", bufs=1, space="PSUM") as mps, \
             tc.tile_pool(name="tpool", bufs=3) as tp:
            for n in range(NT):
                pt = [mps.tile([128, 512], F32, tag=f"ps{ib}", name=f"pt{ib}") for ib in range(4)]
                sc_ps = mps.tile([1, 512], F32, tag="sc_ps")
                for jg in range(JC // 4):
                    vth4 = vp.tile([128, 4 * 512], BF16, tag="vth4", name="vth4")
                    nc.sync.dma_start(
                        vth4[:].rearrange("p (g c) -> p g c", g=4),
                        vh[jg * 512:(jg + 1) * 512, n * 512:(n + 1) * 512]
                            .rearrange("(g p) c -> p g c", p=128))
                    vtl4 = vp.tile([128, 4 * 512], BF16, tag="vtl4", name="vtl4")
                    nc.scalar.dma_start(
                        vtl4[:].rearrange("p (g c) -> p g c", g=4),
                        vl[jg * 512:(jg + 1) * 512, n * 512:(n + 1) * 512]
                            .rearrange("(g p) c -> p g c", p=128))
                    for g in range(4):
                        jc = jg * 4 + g
                        vth = vth4[:, g * 512:(g + 1) * 512]
                        vtl = vtl4[:, g * 512:(g + 1) * 512]
                        for ib in range(4):
                            wh = w1h_all[:, jc * ISLC + ib * 128:jc * ISLC + (ib + 1) * 128]
                            wl = w1l_all[:, jc * ISLC + ib * 128:jc * ISLC + (ib + 1) * 128]
                            st = (jc == 0)
                            nc.tensor.matmul(pt[ib][:], wh, vth, start=st, stop=False)
                            nc.tensor.matmul(pt[ib][:], wh, vtl, start=False, stop=False)
                            nc.tensor.matmul(pt[ib][:], wl, vth, start=False, stop=False)
                for ib in range(4):
                    nc.tensor.matmul(pt[ib][:], lhsx[:, ib * 128:(ib + 1) * 128],
                                     rhsx[n][:], start=False, stop=True)
                    tanh_sb = tp.tile([128, 512], F32, tag="tanh")
                    nc.scalar.activation(tanh_sb[:], pt[ib][:], AF.Tanh)
                    nc.tensor.matmul(sc_ps[:], vwt_sb[ib][:], tanh_sb[:],
                                     start=(ib == 0), stop=(ib == 3))
                sc_sb = tp.tile([1, 512], F32, tag="sc_sb", name="sc_sb")
                nc.vector.tensor_scalar(
                    sc_sb[:], sc_ps[:], vb8[0:1, 0:1], None, op0=ALU.add)
                nc.sync.dma_start(arin[0:1, n * 512:(n + 1) * 512], sc_sb[:])

        w1pool.release()

        # ================= allreduce =================
        if use_collective:
            nc.gpsimd.collective_compute(
                "AllReduce", ALU.add,
                replica_groups=[list(range(NCORES))],
                ins=[arin[:].opt()],
                outs=[arout[:].opt()],
            )
        else:
            nc.gpsimd.dma_start(arout[:], arin[:])
        score_full = cst.tile([B, S], F32, tag="score_full")
        nc.gpsimd.dma_start(score_full[:],
                            arout[0:1, 0:BS].squeeze(0).rearrange("(b s) -> b s", b=B))

        # ================= top-k mask + sigmoid + normalize =================
        scratch = cst.tile([B, S], F32, tag="scratch")
        nc.vector.tensor_copy(scratch[:], score_full[:])
        mx = cst.tile([B, 8], F32, tag="mx")
        nrounds = (TOPK + 7) // 8
        for r in range(nrounds):
            nc.vector.max(out=mx[:], in_=scratch[:])
            rem = TOPK - 8 * r
            if rem < 8:
                nc.vector.memset(mx[:, rem:], NEG)
            nc.vector.match_replace(out=scratch[:], in_to_replace=mx[:],
                                    in_values=scratch[:], imm_value=NEG)
        s_kept = cst.tile([B, S], F32, tag="s_kept")
        nc.vector.scalar_tensor_tensor(
            s_kept[:], scratch[:], -1.0e29, score_full[:],
            op0=ALU.is_lt, op1=ALU.mult)
        nc.sync.dma_start(sc_o[:, :], s_kept[:])

        sig = cst.tile([B, S], F32, tag="sig")
        nc.scalar.activation(sig[:], s_kept[:], AF.Sigmoid)
        ones20 = cst.tile([B, 1], F32, tag="ones20")
        nc.vector.memset(ones20[:], 1.0)
        ps_cs = pp.tile([1, S], F32, tag="ppx")
        nc.tensor.matmul(ps_cs[:], ones20[:], sig[:], start=True, stop=True)
        den_r = cst.tile([1, S], F32, tag="den_r")
        nc.vector.reciprocal(den_r[:], ps_cs[:])
        ones1 = cst.tile([1, B], F32, tag="ones1")
        nc.vector.memset(ones1[:], 1.0)
        ps_rep = pp.tile([B, S], F32, tag="ppx")
        nc.tensor.matmul(ps_rep[:], ones1[:], den_r[:], start=True, stop=True)
        aw = cst.tile([B, S], F32, tag="aw")
        nc.vector.tensor_mul(aw[:], sig[:], ps_rep[:])
        nc.sync.dma_start(aw_o[:, :], aw[:])

        # aw_flat via DRAM bounce (reuse aw_o), then replicate to 128 partitions
        aw_flat = cst.tile([1, BSP], F32, tag="aw_flat")
        nc.vector.memset(aw_flat[:], 0.0)
        nc.sync.dma_start(aw_flat[0:1, 0:BS],
                          aw_o[:, :].rearrange("b s -> (b s)").unsqueeze(0))
        ones_col = cst.tile([1, 128], F32, tag="ones_col")
        nc.vector.memset(ones_col[:], 1.0)
        aw_b = cst.tile([128, BSP], F32, tag="aw_b")
        for n in range(NT):
            ps_awb = pp.tile([128, 512], F32, tag="ppx", name="ps_awb")
            nc.tensor.matmul(ps_awb[:], ones_col[:],
                             aw_flat[0:1, n * 512:(n + 1) * 512],
                             start=True, stop=True)
            nc.scalar.activation(aw_b[:, n * 512:(n + 1) * 512], ps_awb[:], AF.Copy)

        # ================= context =================
        ctx_final = cst.tile([B, ISLC], F32, tag="ctx_final")
        with tc.tile_pool(name="cpool", bufs=2) as cp, \
             tc.tile_pool(name="cps", bufs=2, space="PSUM") as cps:
            for hc in range(4):
                ch_t = cp.tile([128, BSP], BF16, tag="ch")
                nc.sync.dma_start(ch_t[:], vch[hc * 128:(hc + 1) * 128, :])
                cl_t = cp.tile([128, BSP], BF16, tag="cl")
                nc.scalar.dma_start(cl_t[:], vcl[hc * 128:(hc + 1) * 128, :])
                comb = cp.tile([128, BSP], F32, tag="comb")
                nc.vector.tensor_add(comb[:], ch_t[:], cl_t[:])
                nc.vector.tensor_mul(comb[:], comb[:], aw_b[:])
                cpart = cp.tile([128, B], F32, tag="cpart")
                nc.vector.reduce_sum(
                    out=cpart[:].unsqueeze(-1),
                    in_=comb[0:128, 0:BS].rearrange("p (b s) -> p b s", b=B),
                    axis=mybir.AxisListType.X)
                tr_ps = cps.tile([B, 128], F32, tag="tr_ps")
                nc.tensor.transpose(tr_ps[:], cpart[:], ident[:])
                nc.scalar.activation(ctx_final[:, hc * 128:(hc + 1) * 128],
                                     tr_ps[:], AF.Copy)
        nc.sync.dma_start(ctx_o[:, :], ctx_final[:])

    nc.compile()
    return nc


def _split(x):
    x = np.asarray(x, np.float32)
    hi = x.astype(ml_dtypes.bfloat16)
    lo = (x - hi.astype(np.float32)).astype(ml_dtypes.bfloat16)
    return np.ascontiguousarray(hi), np.ascontiguousarray(lo)


def _prep(query, values, prev_att, W1_w, W1_b, W2_w, W2_b, V_w, V_b,
          conv_w, locproj_w):
    K = (conv_w.shape[2] - 1) // 2
    vT = np.zeros((H, BSP), np.float32)
    vT[:, :BS] = np.asarray(values, np.float32).reshape(BS, H).T
    vh, vl = _split(vT)
    W1T = np.asarray(W1_w, np.float32).T            # [j, i]
    W2T = np.asarray(W2_w, np.float32).T
    w1th_f, w1tl_f = _split(W1T)
    w2th_f, w2tl_f = _split(W2T)
    qth_f, qtl_f = _split(np.asarray(query, np.float32)[0].T)      # [H, B]
    pvh_f, pvl_f = _split(np.asarray(prev_att, np.float32)[..., 0].T)  # [S, B]
    cw15T = np.ascontiguousarray(np.asarray(conv_w, np.float32)[:, :, K].T)
    cwh_f, cwl_f = _split(cw15T)
    lph_f, lpl_f = _split(np.asarray(locproj_w, np.float32)[:, 0][None, :])
    onehot = np.zeros((B + 2, BSP), np.float32)
    for b in range(B):
        onehot[b, b * S:(b + 1) * S] = 1.0
    onehot[B:B + 2, :] = 1.0
    oh_bf = onehot.astype(ml_dtypes.bfloat16)
    W1b = np.asarray(W1_b, np.float32)[None, :]
    W2b = np.asarray(W2_b, np.float32)[None, :]
    Vw = np.asarray(V_w, np.float32)
    Vb = np.asarray(V_b, np.float32).reshape(1, 1)

    in_maps = []
    for c in range(NCORES):
        sl = slice(c * ISLC, (c + 1) * ISLC)
        in_maps.append({
            "vh": vh, "vl": vl,
            "w1th": np.ascontiguousarray(w1th_f[:, sl]),
            "w1tl": np.ascontiguousarray(w1tl_f[:, sl]),
            "w2th": np.ascontiguousarray(w2th_f[:, sl]),
            "w2tl": np.ascontiguousarray(w2tl_f[:, sl]),
            "qth": qth_f, "qtl": qtl_f,
            "pvh": pvh_f, "pvl": pvl_f,
            "cwh": cwh_f, "cwl": cwl_f,
            "lph": np.ascontiguousarray(lph_f[:, sl]),
            "lpl": np.ascontiguousarray(lpl_f[:, sl]),
            "w1b": np.ascontiguousarray(W1b[:, sl]),
            "w2b": np.ascontiguousarray(W2b[:, sl]),
            "vwt": np.ascontiguousarray(Vw[0, sl][:, None]),
            "vb": Vb,
            "oh": oh_bf,
            "vch": np.ascontiguousarray(vh[sl, :]),
            "vcl": np.ascontiguousarray(vl[sl, :]),
        })
    return in_maps


def kernel(**inputs):
    if "nc" not in _cache:
        _cache["nc"] = _build()  # compiled once, reused across calls
    nc = _cache["nc"]
    in_maps = _prep(**inputs)
    last = None
    for _attempt in range(3):  # transient NRT/axon errors on cold devices
        try:
            r = run_bass_kernel_spmd(nc, in_maps, core_ids=list(range(NCORES)))
            break
        except Exception as ex:
            last = ex
    else:
        raise last
    ctx = np.concatenate([r.results[c]["ctx_o"] for c in range(NCORES)], axis=1)
    aw = r.results[0]["aw_o"].reshape(B, S, 1)
    sc = r.results[0]["sc_o"].reshape(B, S, 1)
    return (ctx.astype(np.float32), aw.astype(np.float32),
            sc.astype(np.float32))
